# revision 19
# baseline (speedup 1.0000x reference)
"""MoH (mixture-of-heads) attention kernel for 8 Trainium2 NeuronCores.

Problem (hardcoded shapes): x [2, 2048, 1024], 16 heads x 64 dim.
  q,k,v = x @ W{q,k,v}.T + b      -> [B, H, N, hd]
  q     = q / ||q||; q = (q + query_embedding) * softplus(temperature)
  h     = softmax(q k^T / sqrt(hd)) v  -> [B, N, 1024]
  gates = softmax(h @ Wr.T + br); top-3 mask; sw = softmax(h @ Ws.T + bs)
  g     = 2*sw0 + 6*sw1*sum(top3(gates))      (per-token scalar)
  out   = (h * g) @ Wp.T + bp

Sharding: token-parallel. Core c (of 8) owns batch b=c//4 and its token
block [512*(c%4), 512*(c%4)+512).  Each core projects q/k/v for its own
512 tokens; k (channel-major) and v (key-major, 65-wide head slots with a
built-in ones column for the softmax denominator) are AllGathered within
the 4-core group of the same batch; attention (512 queries x 2048 keys),
routing gates and the output projection then run fully locally.

All matmuls are bf16 (PSUM fp32).  Attention uses PE-array ROW TILING
(64x128 mode): QK runs head-even on tile (0,0) and head-odd on tile
(64,0) concurrently (each head's 64 q-channels live on one partition
half); PV splits the 128-key contraction across the two tiles and the
partials are summed on DVE.  A tiled pair retires in ~259 ns vs 2x216
serial.  Softmax skips max-subtraction (logits are O(1)); the exp runs
on the scalar engine per k-tile as [128,2,512] (both heads' scores in
one PSUM tile, distinct banks per PE tile); the denominator comes free
from a ones column in the v slots.  Per-head reciprocal + partial-copy +
v-bias run on gpsimd; the 1/den row is partition-broadcast with a
stride-0 DMA.  8 PSUM banks exactly: 2x[128,2,512] score tiles (double
buffer) + 4x[65,512] PV accumulators.
"""

import numpy as np
from contextlib import ExitStack

import concourse.bacc as bacc
import concourse.tile as tile
from concourse import mybir
from concourse.bass_utils import run_bass_kernel_spmd
import ml_dtypes

BF16NP = ml_dtypes.bfloat16

F32 = mybir.dt.float32
F32R = mybir.dt.float32r
BF16 = mybir.dt.bfloat16
AF = mybir.ActivationFunctionType
ALU = mybir.AluOpType
AX = mybir.AxisListType

B, N, D = 2, 2048, 1024
H, HD = 16, 64
NCORE = 8
TOK = 512                      # tokens per core
KT = N // 128                  # 16 k-token tiles per batch
GROUPS = [[0, 1, 2, 3], [4, 5, 6, 7]]
KSZ = 2 * 128 * TOK            # k payload elems per group (2 s-slots)
VSZ = 128 * 4 * 260            # v payload elems per group (65-wide slots)
SH = KSZ + VSZ                 # one core's per-group collective payload


def build_nc():
    nc = bacc.Bacc(None, target_bir_lowering=False, num_devices=NCORE)

    xT = nc.declare_dram_parameter("xT", [128, 8, TOK], BF16, isOutput=False)
    wqT = nc.declare_dram_parameter("wqT", [128, 8, D], BF16, isOutput=False)
    wkT = nc.declare_dram_parameter("wkT", [128, 8, D], BF16, isOutput=False)
    wvT = nc.declare_dram_parameter("wvT", [128, 8, D], BF16, isOutput=False)
    wpT = nc.declare_dram_parameter("wpT", [D, D], BF16, isOutput=False)
    wrsT = nc.declare_dram_parameter("wrsT", [D, 17], F32R, isOutput=False)
    bq = nc.declare_dram_parameter("bq", [D], F32, isOutput=False)
    bv = nc.declare_dram_parameter("bv", [D], F32, isOutput=False)
    bp = nc.declare_dram_parameter("bp", [D], F32, isOutput=False)
    brs = nc.declare_dram_parameter("brs", [17], F32, isOutput=False)
    temp16 = nc.declare_dram_parameter("temp16", [16], F32, isOutput=False)
    qe = nc.declare_dram_parameter("qe", [H, HD], F32, isOutput=False)
    msel = nc.declare_dram_parameter("msel", [8, 128, 16], BF16, isOutput=False)
    esel = nc.declare_dram_parameter("esel", [8, 16, 128], F32R, isOutput=False)
    ident = nc.declare_dram_parameter("ident", [128, 128], F32, isOutput=False)
    ones_r = nc.declare_dram_parameter("ones_r", [128, HD], F32R, isOutput=False)
    out = nc.declare_dram_parameter("out", [TOK, D], F32, isOutput=True)

    with tile.TileContext(nc) as tc, ExitStack() as ctx:
        const = ctx.enter_context(tc.tile_pool(name="const", bufs=1))
        psum = ctx.enter_context(tc.tile_pool(name="psum", bufs=2, space="PSUM"))
        dram = ctx.enter_context(tc.tile_pool(name="dram", bufs=1, space="DRAM"))
        work = ctx.enter_context(tc.tile_pool(name="work", bufs=1))
        kv = ctx.enter_context(tc.tile_pool(name="kv", bufs=2))

        # ---- stage 0: x + weights (upfront, before the collectives own
        # the DMA rings) + constants --------------------------------------
        xT16 = work.tile([128, 8, TOK], BF16)
        nc.sync.dma_start(out=xT16, in_=xT[:, :, :])
        wk_sb = work.tile([128, 8, D], BF16)
        nc.scalar.dma_start(out=wk_sb, in_=wkT[:, :, :])
        wv_sb = work.tile([128, 8, D], BF16)
        nc.sync.dma_start(out=wv_sb, in_=wvT[:, :, :])
        wq_sb = work.tile([128, 8, D], BF16)
        nc.scalar.dma_start(out=wq_sb, in_=wqT[:, :, :])

        bq_ch = const.tile([128, 8], F32)
        nc.gpsimd.dma_start(out=bq_ch, in_=bq.rearrange("(s p) -> p s", p=128))
        bv_ch = const.tile([128, 8], F32)
        nc.gpsimd.dma_start(out=bv_ch, in_=bv.rearrange("(s p) -> p s", p=128))
        msel_sb = const.tile([128, 8, 16], BF16)
        nc.gpsimd.dma_start(out=msel_sb, in_=msel.rearrange("s p h -> p s h"))
        esel_sb = const.tile([16, 8, 128], F32R)
        nc.gpsimd.dma_start(out=esel_sb, in_=esel.rearrange("s h m -> h s m"))

        # softplus(t) = ln(1 + exp(t)) -- keeps ACT on one table set (exp/ln)
        temp_sb = const.tile([16, 1], F32)
        nc.gpsimd.dma_start(out=temp_sb, in_=temp16[:, None])
        sp8 = const.tile([16, 1], F32)
        nc.scalar.activation(sp8, temp_sb, AF.Exp)
        nc.vector.tensor_scalar_add(sp8, sp8, 1.0)
        nc.scalar.activation(sp8, sp8, AF.Ln)
        nc.vector.tensor_scalar_mul(sp8, sp8, 0.125)
        qe_sb = const.tile([16, HD], F32)
        nc.gpsimd.dma_start(out=qe_sb, in_=qe[:, :])
        qe_sp16 = const.tile([16, HD], F32)
        nc.vector.tensor_tensor(qe_sp16, qe_sb,
                                sp8[:, 0:1].to_broadcast([16, HD]), ALU.mult)
        # reshape [16,64](head-major) -> [128,8](channel-major) via DRAM
        qe_scr = dram.tile([D], F32)
        nc.sync.dma_start(out=qe_scr.rearrange("(h d) -> h d", h=16), in_=qe_sp16)
        qe_ch = const.tile([128, 8], F32)
        nc.gpsimd.dma_start(out=qe_ch, in_=qe_scr.rearrange("(s p) -> p s", p=128))

        ones_sb = const.tile([128, HD], F32R)
        nc.gpsimd.dma_start(out=ones_sb, in_=ones_r[:, :])

        cc_in = [dram.tile([SH], BF16, name=f"ccin{g}") for g in range(4)]
        cc_out = [dram.tile([4 * SH], BF16, name=f"ccout{g}") for g in range(4)]

        # tiny dummy collectives: the CC engine only starts mesh N once
        # later trigger doorbells arrive, so flush the pipeline at both ends
        dmy_in = [dram.tile([64], BF16, name=f"dmyi{i}") for i in range(4)]
        dmy_out = [dram.tile([256], BF16, name=f"dmyo{i}") for i in range(4)]
        mflat = msel.rearrange("s p h -> (s p h)")

        def dummy_ag(i):
            nc.gpsimd.dma_start(out=dmy_in[i], in_=mflat[0:64])
            nc.gpsimd.collective_compute(
                "AllGather", ALU.bypass, replica_groups=GROUPS,
                ins=[dmy_in[i].opt()], outs=[dmy_out[i].opt()])

        dummy_ag(0)
        dummy_ag(1)

        # persistent v-payload staging tiles with baked-in ones columns
        vc = []
        for i in range(4):
            t = work.tile([128, 2, 4, 65], BF16, name=f"vc{i}")
            nc.vector.memset(t, 1.0)     # ones col survives the data copy
            vc.append(t)

        # ---- stage 1: k/v projections + pipelined AllGathers -----------
        def kv_proj(g):
            pk = psum.tile([128, 2, TOK], F32, tag="mm", name=f"pk{g}")
            for ks in range(8):
                for half in range(2):
                    nc.tensor.matmul(
                        pk[:, half, :],
                        wk_sb[:, ks, 256 * g + 128 * half:256 * g + 128 * half + 128],
                        xT16[:, ks, :],
                        start=(ks == 0), stop=(ks == 7))
            # NOTE: k bias dropped -- q.(k+bk) differs from q.k by a
            # per-query constant, which cancels in the softmax exactly
            kc = kv.tile([128, 2, TOK], BF16, tag="kc", name=f"kc{g}")
            nc.vector.tensor_copy(kc, pk)
            nc.gpsimd.dma_start(
                out=cc_in[g][0:KSZ].rearrange("(s p t) -> p s t", p=128, t=TOK),
                in_=kc)

            pvt = [psum.tile([128, 2, TOK], F32, tag="mm", name=f"pvt{g}_{i}")
                   for i in range(2)]
            for ks in range(8):
                for mt in range(4):
                    nc.tensor.matmul(pvt[mt // 2][:, mt % 2, 0:256],
                                     xT16[:, ks, 128 * mt:128 * mt + 128],
                                     wv_sb[:, ks, 256 * g:256 * g + 256],
                                     start=(ks == 0), stop=(ks == 7))
            ccv = cc_in[g][KSZ:SH].rearrange("(p m o) -> p m o", m=4, o=260)
            for i in range(2):
                vci = vc[2 * (g % 2) + i]
                nc.vector.tensor_copy(
                    vci[:, :, :, 0:64],
                    pvt[i][:, :, 0:256].rearrange("p m (h d) -> p m h d", h=4))
                nc.gpsimd.dma_start(
                    out=ccv[:, 2 * i:2 * i + 2, :],
                    in_=vci.rearrange("p m h d -> p m (h d)"))

            nc.gpsimd.collective_compute(
                "AllGather", ALU.bypass, replica_groups=GROUPS,
                ins=[cc_in[g].opt()], outs=[cc_out[g].opt()])

        # receiver-side slabs (double-buffered by group parity)
        kz = [work.tile([128, 2, N], BF16, name=f"kz{i}") for i in range(2)]
        v2 = [work.tile([128, KT, 260], BF16, name=f"v2{i}") for i in range(2)]

        def kv_recv(g):
            buf = g % 2
            for j in range(4):
                kj = cc_out[g][j * SH:j * SH + KSZ].rearrange(
                    "(s p t) -> p s t", p=128, t=TOK)
                nc.sync.dma_start(out=kz[buf][:, :, TOK * j:TOK * j + TOK],
                                  in_=kj)
                vj = cc_out[g][j * SH + KSZ:(j + 1) * SH].rearrange(
                    "(p m o) -> p m o", m=4, o=260)
                nc.sync.dma_start(out=v2[buf][:, 4 * j:4 * j + 4, :], in_=vj)

        # all 4 payload projections + collective triggers FIRST: the mesh
        # only starts once every payload DMA has landed (shared sem target),
        # so nothing may delay them
        kv_proj(0)
        kv_proj(1)
        kv_proj(2)
        kv_proj(3)
        dummy_ag(2)
        dummy_ag(3)

        # q projection + q-norm while the AllGathers are in flight
        q_sb = work.tile([128, 8, TOK], BF16)
        for sp_ in range(4):
            pq = psum.tile([128, 2, TOK], F32, tag="mm", name=f"pq{sp_}")
            for ks in range(8):
                for half in range(2):
                    nc.tensor.matmul(
                        pq[:, half, :],
                        wq_sb[:, ks,
                              256 * sp_ + 128 * half:256 * sp_ + 128 * half + 128],
                        xT16[:, ks, :],
                        start=(ks == 0), stop=(ks == 7))
            nc.vector.tensor_tensor(
                q_sb[:, 2 * sp_:2 * sp_ + 2, :], pq,
                bq_ch[:, 2 * sp_:2 * sp_ + 2, None].to_broadcast(
                    [128, 2, TOK]), ALU.add)

        # q-norm + scale + query-embedding, all channel-major
        pss = psum.tile([16, TOK], F32, tag="pv", bufs=4)
        for s in range(8):
            sq_t = kv.tile([128, TOK], BF16, tag="sq", name=f"sq{s}")
            nc.vector.tensor_mul(sq_t, q_sb[:, s, :], q_sb[:, s, :])
            nc.tensor.matmul(pss, msel_sb[:, s, :], sq_t,
                             start=(s == 0), stop=(s == 7))
        # rsqrt(ss) = exp(-0.5 * ln(ss)) -- same exp/ln ACT table set
        sqs = const.tile([16, TOK], F32)
        nc.scalar.activation(sqs, pss, AF.Ln)
        rr = const.tile([16, TOK], F32)
        nc.scalar.activation(rr, sqs, AF.Exp, scale=-0.5)
        rs_sp = const.tile([16, TOK], F32R)
        nc.vector.tensor_tensor(rs_sp, rr, sp8[:, 0:1].to_broadcast([16, TOK]),
                                ALU.mult)
        for s in range(8):
            pb = psum.tile([128, TOK], F32, tag="pv", bufs=4, name=f"pb{s}")
            nc.tensor.matmul(pb, esel_sb[:, s, :], rs_sp, start=True, stop=True)
            nc.vector.tensor_mul(q_sb[:, s, :], q_sb[:, s, :], pb)
            nc.vector.tensor_tensor(
                q_sb[:, s, :], q_sb[:, s, :],
                qe_ch[:, s:s + 1].to_broadcast([128, TOK]), ALU.add)

        kv_recv(0)
        kv_recv(1)

        # prefetch Wp + output constants during attention
        wp_sb = const.tile([128, 8, D], BF16)
        nc.scalar.dma_start(out=wp_sb,
                            in_=wpT.rearrange("(s p) co -> p s co", p=128))
        bp_rep = const.tile([128, D], F32)
        nc.gpsimd.dma_start(out=bp_rep, in_=bp[None, :].to_broadcast([128, D]))
        brs_sb = const.tile([17, 1], F32)
        nc.gpsimd.dma_start(out=brs_sb, in_=brs[:, None])
        ident_sb = const.tile([128, 128], F32)
        nc.gpsimd.dma_start(out=ident_sb, in_=ident[:, :])
        w_rs = const.tile([128, 8, 17], F32R)
        nc.gpsimd.dma_start(out=w_rs,
                            in_=wrsT.rearrange("(s p) o -> p s o", p=128))
        w_rs16 = const.tile([128, 8, 17], BF16)
        nc.vector.tensor_copy(w_rs16, w_rs)

        # ---- stage 2: attention, row-tiled 64x128 ----------------------
        # group g's receive into slab buf g%2 must be ISSUED after the
        # attention of group g-2 (program order = tile version order)
        hT16 = work.tile([128, 8, TOK], BF16)    # packed channel-major h
        for s in range(8):                       # head pair (2s, 2s+1)
            g = s // 2
            sg = s % 2
            if sg == 0 and 2 <= g + 1 <= 3:
                kv_recv(g + 1)   # overlaps group g's attention; WAR-safe
            if sg == 0:
                hsg = kv.tile([65, 4, TOK], F32R, tag="hs", name=f"hsg{g}")
            buf = g % 2
            kzs = kz[buf]
            v2s = v2[buf]
            pv_acc = [psum.tile([65, TOK], F32, tag="pv", bufs=4,
                                name=f"pvacc{s}_{i}") for i in range(4)]
            ets = [None] * KT

            def pv_step(kt):
                # PV for k-tile kt (key halves on PE tiles T0/T8)
                et = ets[kt]
                for par in range(2):             # head parity
                    vsl = 65 * (2 * sg + par)
                    nc.tensor.matmul(pv_acc[2 * par],
                                     v2s[0:64, kt, vsl:vsl + 65],
                                     et[0:64, par, :],
                                     start=(kt == 0), stop=(kt == KT - 1))
                    nc.tensor.matmul(pv_acc[2 * par + 1],
                                     v2s[64:128, kt, vsl:vsl + 65],
                                     et[64:128, par, :],
                                     start=(kt == 0), stop=(kt == KT - 1))

            SKEW = 3        # PV trails QK/exp so the PE never waits on ACT
            for kt in range(KT):
                sc = psum.tile([128, 2, TOK], F32, tag="mm", name=f"sc{s}_{kt}")
                nc.tensor.matmul(sc[:, 0, :],
                                 kzs[0:64, sg, 128 * kt:128 * kt + 128],
                                 q_sb[0:64, s, :], start=True, stop=True)
                nc.tensor.matmul(sc[:, 1, :],
                                 kzs[64:128, sg, 128 * kt:128 * kt + 128],
                                 q_sb[64:128, s, :], start=True, stop=True)
                et = kv.tile([128, 2, TOK], BF16, tag="et", bufs=SKEW + 2,
                             name=f"et{s}_{kt}")
                nc.scalar.activation(et, sc, AF.Exp)
                ets[kt] = et
                if kt >= SKEW:
                    pv_step(kt - SKEW)
            for kt in range(KT - SKEW, KT):
                pv_step(kt)

            # normalize: h = (pvT0 + pvT8)[0:64] / den + bv
            # (gpsimd has no PSUM port: psum-touching ops stay on DVE)
            for par in range(2):
                c8 = kv.tile([65, TOK], F32R, tag="c8", name=f"c8_{s}_{par}")
                nc.vector.tensor_copy(c8, pv_acc[2 * par + 1])
                nc.vector.tensor_tensor(hsg[:, 2 * sg + par, :],
                                        pv_acc[2 * par], c8, ALU.add)
            if sg == 1:
                # one batched reciprocal for the whole group's denominators
                # (DVE reciprocal has a ~2.9us fixed cost per instruction)
                rc = kv.tile([1, 4, TOK], F32R, tag="rc", name=f"rc_{g}")
                with nc.allow_low_precision(reason="softmax denominator"):
                    nc.vector.reciprocal(rc, hsg[64:65, :, :])
                for gh in range(4):              # head 4g+gh
                    s2 = 2 * g + gh // 2
                    par = gh % 2
                    pgb = psum.tile([64, TOK], F32, tag="pv", bufs=4,
                                    name=f"pgb_{g}_{gh}")
                    nc.tensor.matmul(pgb, ones_sb[0:1, :], rc[0:1, gh, :],
                                     start=True, stop=True)
                    hview = hT16[64 * par:64 * par + 64, s2, :]
                    nc.vector.tensor_mul(hview, hsg[0:64, gh, :], pgb)
                    nc.gpsimd.tensor_tensor(
                        hview, hview,
                        bv_ch[64 * par:64 * par + 64, s2:s2 + 1].to_broadcast(
                            [64, TOK]), ALU.add)

        # ---- stage 3: routing gates -> per-token scalar g --------------
        prs = psum.tile([17, TOK], F32, tag="pv", bufs=4)
        for s in range(8):
            nc.tensor.matmul(prs, w_rs16[:, s, :], hT16[:, s, :],
                             start=(s == 0), stop=(s == 7))
        rs_sb = const.tile([17, TOK], F32)
        nc.vector.tensor_tensor(rs_sb, prs,
                                brs_sb[:, 0:1].to_broadcast([17, TOK]), ALU.add)
        lg_t = const.tile([128, 4, 17], F32)
        for c4 in range(4):
            pt_ = psum.tile([128, 17], F32, tag="pv", bufs=4, name=f"pt{c4}")
            nc.tensor.transpose(pt_, rs_sb[:, 128 * c4:128 * c4 + 128],
                                ident_sb[0:17, 0:17])
            nc.vector.tensor_copy(lg_t[:, c4, :], pt_)

        e15 = const.tile([128, 4, 15], F32)
        nc.scalar.activation(e15, lg_t[:, :, 0:15], AF.Exp)
        e2 = const.tile([128, 4, 2], F32)
        nc.scalar.activation(e2, lg_t[:, :, 15:17], AF.Exp)
        s15 = const.tile([128, 4, 1], F32)
        nc.vector.tensor_reduce(s15, e15, AX.X, ALU.add)
        s2 = const.tile([128, 4, 1], F32)
        nc.vector.tensor_reduce(s2, e2, AX.X, ALU.add)
        m1 = const.tile([128, 4, 1], F32)
        nc.vector.tensor_reduce(m1, e15, AX.X, ALU.max)
        msk = const.tile([128, 4, 15], F32)
        nc.vector.tensor_tensor(msk, e15, m1.to_broadcast([128, 4, 15]), ALU.is_ge)
        e15b = const.tile([128, 4, 15], F32)
        nc.vector.scalar_tensor_tensor(e15b, msk, -1e30, e15, ALU.mult, ALU.add)
        m2 = const.tile([128, 4, 1], F32)
        nc.vector.tensor_reduce(m2, e15b, AX.X, ALU.max)
        nc.vector.tensor_tensor(msk, e15b, m2.to_broadcast([128, 4, 15]), ALU.is_ge)
        nc.vector.scalar_tensor_tensor(e15b, msk, -1e30, e15b, ALU.mult, ALU.add)
        m3 = const.tile([128, 4, 1], F32)
        nc.vector.tensor_reduce(m3, e15b, AX.X, ALU.max)
        nc.vector.tensor_add(m1, m1, m2)
        nc.vector.tensor_add(m1, m1, m3)       # m1 = top3 sum of e15
        nc.vector.reciprocal(s15, s15)
        nc.vector.reciprocal(s2, s2)
        ga = const.tile([128, 4, 1], F32)
        nc.vector.tensor_mul(ga, e2[:, :, 0:1], s2)
        gb = const.tile([128, 4, 1], F32)
        nc.vector.tensor_mul(gb, e2[:, :, 1:2], s2)
        nc.vector.tensor_mul(gb, gb, m1)
        nc.vector.tensor_mul(gb, gb, s15)
        nc.vector.tensor_scalar_mul(gb, gb, 6.0)
        gsc = const.tile([128, 4, 1], F32)
        nc.vector.scalar_tensor_tensor(gsc, ga, 2.0, gb, ALU.mult, ALU.add)

        # ---- stage 4: output projection --------------------------------
        for nt in range(2):
            po = [psum.tile([128, 2, TOK], F32, tag="mm", name=f"po{nt}_{i}")
                  for i in range(2)]
            for s in range(8):
                for mt in range(4):
                    nc.tensor.matmul(
                        po[mt // 2][:, mt % 2, :],
                        hT16[:, s, 128 * mt:128 * mt + 128],
                        wp_sb[:, s, TOK * nt:TOK * nt + TOK],
                        start=(s == 0), stop=(s == 7))
            for mt in range(4):
                ob = kv.tile([128, TOK], F32, tag="ob", bufs=3,
                             name=f"ob{nt}_{mt}")
                nc.vector.tensor_mul(ob, po[mt // 2][:, mt % 2, :],
                                     gsc[:, mt, 0:1].to_broadcast([128, TOK]))
                nc.vector.tensor_add(ob, ob, bp_rep[:, TOK * nt:TOK * nt + TOK])
                nc.sync.dma_start(
                    out=out[128 * mt:128 * mt + 128, TOK * nt:TOK * nt + TOK],
                    in_=ob)

    nc.compile()
    return nc


_NC_CACHE = {}


def _get_nc():
    if "nc" not in _NC_CACHE:
        _NC_CACHE["nc"] = build_nc()
    return _NC_CACHE["nc"]


def _wdev(W):
    """[out,in] weight -> device layout [p, ks, co]: contiguous per
    partition so the upfront DMA is one descriptor per partition."""
    wT = np.asarray(W, np.float32).T.astype(BF16NP)      # [in, out]
    return np.ascontiguousarray(wT.reshape(8, 128, D).transpose(1, 0, 2))


def _host_prep(x, Wq, bq, Wk, bk, Wv, bv, Wp, bp, Wr, br, Ws, bs,
               temperature, query_embedding):
    f32 = np.float32
    xf = np.ascontiguousarray(x, dtype=f32).reshape(B * N, D)
    shared = {
        "wqT": _wdev(Wq), "wkT": _wdev(Wk), "wvT": _wdev(Wv),
        "wpT": np.ascontiguousarray(np.asarray(Wp, f32).T.astype(BF16NP)),
        "wrsT": np.ascontiguousarray(
            np.concatenate([np.asarray(Wr, f32), np.asarray(Ws, f32)], 0).T),
        "bq": np.ascontiguousarray(bq, f32),
        "bv": np.ascontiguousarray(bv, f32), "bp": np.ascontiguousarray(bp, f32),
        "brs": np.concatenate([np.asarray(br, f32), np.asarray(bs, f32)]),
        "temp16": np.ascontiguousarray(np.asarray(temperature, f32).reshape(H)),
        "qe": np.ascontiguousarray(np.asarray(query_embedding, f32).reshape(H, HD)),
        "ident": np.eye(128, dtype=f32),
        "ones_r": np.ones((128, HD), dtype=f32),
    }
    ch = np.arange(D)
    head_of_ch = ch // HD
    msel = np.zeros((8, 128, 16), BF16NP)
    esel = np.zeros((8, 16, 128), f32)
    for s in range(8):
        hh = head_of_ch[128 * s:128 * s + 128]
        msel[s, np.arange(128), hh] = 1.0
        esel[s, hh, np.arange(128)] = 1.0
    shared["msel"] = msel
    shared["esel"] = esel

    in_maps = []
    for c in range(NCORE):
        rows = slice((c // 4) * N + TOK * (c % 4),
                     (c // 4) * N + TOK * (c % 4) + TOK)
        m = dict(shared)
        m["xT"] = np.ascontiguousarray(
            xf[rows].T.reshape(8, 128, TOK).transpose(1, 0, 2).astype(BF16NP))
        in_maps.append(m)
    return in_maps


def kernel(**inputs):
    nc = _get_nc()
    in_maps = _host_prep(**inputs)
    res = run_bass_kernel_spmd(nc, in_maps, core_ids=list(range(NCORE)))
    shards = [res.results[c]["out"] for c in range(NCORE)]
    return np.concatenate(shards, 0).reshape(B, N, D)


# revision 22
# speedup vs baseline: 1.2529x; 1.2529x over previous
"""MoH (mixture-of-heads) attention kernel for 8 Trainium2 NeuronCores.

Problem (hardcoded shapes): x [2, 2048, 1024], 16 heads x 64 dim.
  q,k,v = x @ W{q,k,v}.T + b      -> [B, H, N, hd]
  q     = q / ||q||; q = (q + query_embedding) * softplus(temperature)
  h     = softmax(q k^T / sqrt(hd)) v  -> [B, N, 1024]
  gates = softmax(h @ Wr.T + br); top-3 mask; sw = softmax(h @ Ws.T + bs)
  g     = 2*sw0 + 6*sw1*sum(top3(gates))      (per-token scalar)
  out   = (h * g) @ Wp.T + bp

Sharding: token-parallel. Core c (of 8) owns batch b=c//4 and its token
block [512*(c%4), 512*(c%4)+512).  Each core projects q/k/v for its own
512 tokens; k (channel-major) and v (key-major, 65-wide head slots with a
built-in ones column for the softmax denominator) are AllGathered within
the 4-core group of the same batch; attention (512 queries x 2048 keys),
routing gates and the output projection then run fully locally.

All matmuls are bf16 (PSUM fp32).  Attention uses PE-array ROW TILING
(64x128 mode): QK runs head-even on tile (0,0) and head-odd on tile
(64,0) concurrently (each head's 64 q-channels live on one partition
half); PV splits the 128-key contraction across the two tiles and the
partials are summed on DVE.  A tiled pair retires in ~259 ns vs 2x216
serial.  Softmax skips max-subtraction (logits are O(1)); the exp runs
on the scalar engine per k-tile as [128,2,512] (both heads' scores in
one PSUM tile, distinct banks per PE tile); the denominator comes free
from a ones column in the v slots.  Per-head reciprocal + partial-copy +
v-bias run on gpsimd; the 1/den row is partition-broadcast with a
stride-0 DMA.  8 PSUM banks exactly: 2x[128,2,512] score tiles (double
buffer) + 4x[65,512] PV accumulators.
"""

import numpy as np
from contextlib import ExitStack

import concourse.bacc as bacc
import concourse.tile as tile
from concourse import mybir
from concourse.bass_utils import run_bass_kernel_spmd
import ml_dtypes

BF16NP = ml_dtypes.bfloat16

F32 = mybir.dt.float32
F32R = mybir.dt.float32r
BF16 = mybir.dt.bfloat16
AF = mybir.ActivationFunctionType
ALU = mybir.AluOpType
AX = mybir.AxisListType

B, N, D = 2, 2048, 1024
H, HD = 16, 64
NCORE = 8
TOK = 512                      # tokens per core
KT = N // 128                  # 16 k-token tiles per batch
GROUPS = [[0, 1, 2, 3], [4, 5, 6, 7]]
KSZ = 2 * 128 * TOK            # k payload elems per group (2 s-slots)
VSZ = 128 * 4 * 260            # v payload elems per group (65-wide slots)
SH = KSZ + VSZ                 # one core's per-group collective payload


def build_nc():
    nc = bacc.Bacc(None, target_bir_lowering=False, num_devices=NCORE)

    xT = nc.declare_dram_parameter("xT", [128, 8, TOK], BF16, isOutput=False)
    wqT = nc.declare_dram_parameter("wqT", [128, 8, D], BF16, isOutput=False)
    wkT = nc.declare_dram_parameter("wkT", [128, 8, D], BF16, isOutput=False)
    wvT = nc.declare_dram_parameter("wvT", [128, 8, D], BF16, isOutput=False)
    wpT = nc.declare_dram_parameter("wpT", [D, D], BF16, isOutput=False)
    wrsT = nc.declare_dram_parameter("wrsT", [D, 17], F32R, isOutput=False)
    bq = nc.declare_dram_parameter("bq", [D], F32, isOutput=False)
    bv = nc.declare_dram_parameter("bv", [D], F32, isOutput=False)
    bp = nc.declare_dram_parameter("bp", [D], F32, isOutput=False)
    brs = nc.declare_dram_parameter("brs", [17], F32, isOutput=False)
    temp16 = nc.declare_dram_parameter("temp16", [16], F32, isOutput=False)
    qe = nc.declare_dram_parameter("qe", [H, HD], F32, isOutput=False)
    msel = nc.declare_dram_parameter("msel", [8, 128, 16], BF16, isOutput=False)
    esel = nc.declare_dram_parameter("esel", [8, 16, 128], F32R, isOutput=False)
    ident = nc.declare_dram_parameter("ident", [128, 128], F32, isOutput=False)
    ones_r = nc.declare_dram_parameter("ones_r", [128, HD], F32R, isOutput=False)
    out = nc.declare_dram_parameter("out", [TOK, D], F32, isOutput=True)

    with tile.TileContext(nc) as tc, ExitStack() as ctx:
        const = ctx.enter_context(tc.tile_pool(name="const", bufs=1))
        psum = ctx.enter_context(tc.tile_pool(name="psum", bufs=2, space="PSUM"))
        dram = ctx.enter_context(tc.tile_pool(name="dram", bufs=1, space="DRAM"))
        work = ctx.enter_context(tc.tile_pool(name="work", bufs=1))
        kv = ctx.enter_context(tc.tile_pool(name="kv", bufs=2))

        # ---- stage 0: x + weights (upfront, before the collectives own
        # the DMA rings) + constants --------------------------------------
        xT16 = work.tile([128, 8, TOK], BF16)
        nc.sync.dma_start(out=xT16, in_=xT[:, :, :])
        wk_sb = work.tile([128, 8, D], BF16)
        nc.scalar.dma_start(out=wk_sb, in_=wkT[:, :, :])
        wv_sb = work.tile([128, 8, D], BF16)
        nc.sync.dma_start(out=wv_sb, in_=wvT[:, :, :])
        wq_sb = work.tile([128, 8, D], BF16)
        nc.scalar.dma_start(out=wq_sb, in_=wqT[:, :, :])

        bq_ch = const.tile([128, 8], F32)
        nc.gpsimd.dma_start(out=bq_ch, in_=bq.rearrange("(s p) -> p s", p=128))
        bv_ch = const.tile([128, 8], F32)
        nc.gpsimd.dma_start(out=bv_ch, in_=bv.rearrange("(s p) -> p s", p=128))
        msel_sb = const.tile([128, 8, 16], BF16)
        nc.gpsimd.dma_start(out=msel_sb, in_=msel.rearrange("s p h -> p s h"))
        esel_sb = const.tile([16, 8, 128], F32R)
        nc.gpsimd.dma_start(out=esel_sb, in_=esel.rearrange("s h m -> h s m"))

        # softplus(t) = ln(1 + exp(t)) -- keeps ACT on one table set (exp/ln)
        temp_sb = const.tile([16, 1], F32)
        nc.gpsimd.dma_start(out=temp_sb, in_=temp16[:, None])
        sp8 = const.tile([16, 1], F32)
        nc.scalar.activation(sp8, temp_sb, AF.Exp)
        nc.vector.tensor_scalar_add(sp8, sp8, 1.0)
        nc.scalar.activation(sp8, sp8, AF.Ln)
        nc.vector.tensor_scalar_mul(sp8, sp8, 0.125)
        qe_sb = const.tile([16, HD], F32)
        nc.gpsimd.dma_start(out=qe_sb, in_=qe[:, :])
        qe_sp16 = const.tile([16, HD], F32)
        nc.vector.tensor_tensor(qe_sp16, qe_sb,
                                sp8[:, 0:1].to_broadcast([16, HD]), ALU.mult)
        # reshape [16,64](head-major) -> [128,8](channel-major) via DRAM
        qe_scr = dram.tile([D], F32)
        nc.sync.dma_start(out=qe_scr.rearrange("(h d) -> h d", h=16), in_=qe_sp16)
        qe_ch = const.tile([128, 8], F32)
        nc.gpsimd.dma_start(out=qe_ch, in_=qe_scr.rearrange("(s p) -> p s", p=128))

        ones_sb = const.tile([128, HD], F32R)
        nc.gpsimd.dma_start(out=ones_sb, in_=ones_r[:, :])

        cc_in = [dram.tile([SH], BF16, name=f"ccin{g}") for g in range(4)]
        cc_out = [dram.tile([4 * SH], BF16, name=f"ccout{g}") for g in range(4)]

        # persistent v-payload staging tiles with baked-in ones columns
        vc = []
        for i in range(4):
            t = work.tile([128, 2, 4, 65], BF16, name=f"vc{i}")
            nc.vector.memset(t, 1.0)     # ones col survives the data copy
            vc.append(t)

        # ---- stage 1: k/v projections + pipelined AllGathers -----------
        def kv_proj(g):
            pk = psum.tile([128, 2, TOK], F32, tag="mm", name=f"pk{g}")
            for ks in range(8):
                for half in range(2):
                    nc.tensor.matmul(
                        pk[:, half, :],
                        wk_sb[:, ks, 256 * g + 128 * half:256 * g + 128 * half + 128],
                        xT16[:, ks, :],
                        start=(ks == 0), stop=(ks == 7))
            # NOTE: k bias dropped -- q.(k+bk) differs from q.k by a
            # per-query constant, which cancels in the softmax exactly
            kc = kv.tile([128, 2, TOK], BF16, tag="kc", name=f"kc{g}")
            nc.vector.tensor_copy(kc, pk)
            nc.scalar.dma_start(
                out=cc_in[g][0:KSZ].rearrange("(s p t) -> p s t", p=128, t=TOK),
                in_=kc)

            pvt = [psum.tile([128, 2, TOK], F32, tag="mm", name=f"pvt{g}_{i}")
                   for i in range(2)]
            for ks in range(8):
                for mt in range(4):
                    nc.tensor.matmul(pvt[mt // 2][:, mt % 2, 0:256],
                                     xT16[:, ks, 128 * mt:128 * mt + 128],
                                     wv_sb[:, ks, 256 * g:256 * g + 256],
                                     start=(ks == 0), stop=(ks == 7))
            ccv = cc_in[g][KSZ:SH].rearrange("(p m o) -> p m o", m=4, o=260)
            for i in range(2):
                vci = vc[2 * (g % 2) + i]
                nc.vector.tensor_copy(
                    vci[:, :, :, 0:64],
                    pvt[i][:, :, 0:256].rearrange("p m (h d) -> p m h d", h=4))
                nc.scalar.dma_start(
                    out=ccv[:, 2 * i:2 * i + 2, :],
                    in_=vci.rearrange("p m h d -> p m (h d)"))

            nc.gpsimd.collective_compute(
                "AllGather", ALU.bypass, replica_groups=GROUPS,
                ins=[cc_in[g].opt()], outs=[cc_out[g].opt()])

        # receiver-side slabs (double-buffered by group parity)
        kz = [work.tile([128, 2, N], BF16, name=f"kz{i}") for i in range(2)]
        v2 = [work.tile([128, KT, 260], BF16, name=f"v2{i}") for i in range(2)]

        def kv_recv(g):
            buf = g % 2
            for j in range(4):
                kj = cc_out[g][j * SH:j * SH + KSZ].rearrange(
                    "(s p t) -> p s t", p=128, t=TOK)
                nc.sync.dma_start(out=kz[buf][:, :, TOK * j:TOK * j + TOK],
                                  in_=kj)
                vj = cc_out[g][j * SH + KSZ:(j + 1) * SH].rearrange(
                    "(p m o) -> p m o", m=4, o=260)
                nc.sync.dma_start(out=v2[buf][:, 4 * j:4 * j + 4, :], in_=vj)

        # all 4 payload projections + collective triggers FIRST: the mesh
        # only starts once every payload DMA has landed (shared sem target),
        # so nothing may delay them
        kv_proj(0)
        kv_proj(1)
        kv_proj(2)
        kv_proj(3)

        # q projection + q-norm while the AllGathers are in flight
        q_sb = work.tile([128, 8, TOK], BF16)
        for sp_ in range(4):
            pq = psum.tile([128, 2, TOK], F32, tag="mm", name=f"pq{sp_}")
            for ks in range(8):
                for half in range(2):
                    nc.tensor.matmul(
                        pq[:, half, :],
                        wq_sb[:, ks,
                              256 * sp_ + 128 * half:256 * sp_ + 128 * half + 128],
                        xT16[:, ks, :],
                        start=(ks == 0), stop=(ks == 7))
            nc.vector.tensor_tensor(
                q_sb[:, 2 * sp_:2 * sp_ + 2, :], pq,
                bq_ch[:, 2 * sp_:2 * sp_ + 2, None].to_broadcast(
                    [128, 2, TOK]), ALU.add)

        # q-norm + scale + query-embedding, all channel-major
        pss = psum.tile([16, TOK], F32, tag="pv", bufs=4)
        for s in range(8):
            sq_t = kv.tile([128, TOK], BF16, tag="sq", name=f"sq{s}")
            nc.vector.tensor_mul(sq_t, q_sb[:, s, :], q_sb[:, s, :])
            nc.tensor.matmul(pss, msel_sb[:, s, :], sq_t,
                             start=(s == 0), stop=(s == 7))
        # rsqrt(ss) = exp(-0.5 * ln(ss)) -- same exp/ln ACT table set
        sqs = const.tile([16, TOK], F32)
        nc.scalar.activation(sqs, pss, AF.Ln)
        rr = const.tile([16, TOK], F32)
        nc.scalar.activation(rr, sqs, AF.Exp, scale=-0.5)
        rs_sp = const.tile([16, TOK], F32R)
        nc.vector.tensor_tensor(rs_sp, rr, sp8[:, 0:1].to_broadcast([16, TOK]),
                                ALU.mult)
        for s in range(8):
            pb = psum.tile([128, TOK], F32, tag="pv", bufs=4, name=f"pb{s}")
            nc.tensor.matmul(pb, esel_sb[:, s, :], rs_sp, start=True, stop=True)
            nc.vector.tensor_mul(q_sb[:, s, :], q_sb[:, s, :], pb)
            nc.vector.tensor_tensor(
                q_sb[:, s, :], q_sb[:, s, :],
                qe_ch[:, s:s + 1].to_broadcast([128, TOK]), ALU.add)

        kv_recv(0)
        kv_recv(1)

        # prefetch Wp + output constants during attention
        wp_sb = const.tile([128, 8, D], BF16)
        nc.scalar.dma_start(out=wp_sb,
                            in_=wpT.rearrange("(s p) co -> p s co", p=128))
        bp_rep = const.tile([128, D], F32)
        nc.gpsimd.dma_start(out=bp_rep, in_=bp[None, :].to_broadcast([128, D]))
        brs_sb = const.tile([17, 1], F32)
        nc.gpsimd.dma_start(out=brs_sb, in_=brs[:, None])
        ident_sb = const.tile([128, 128], F32)
        nc.gpsimd.dma_start(out=ident_sb, in_=ident[:, :])
        w_rs = const.tile([128, 8, 17], F32R)
        nc.gpsimd.dma_start(out=w_rs,
                            in_=wrsT.rearrange("(s p) o -> p s o", p=128))
        w_rs16 = const.tile([128, 8, 17], BF16)
        nc.vector.tensor_copy(w_rs16, w_rs)

        # ---- stage 2: attention, row-tiled 64x128 ----------------------
        # group g's receive into slab buf g%2 must be ISSUED after the
        # attention of group g-2 (program order = tile version order)
        hT16 = work.tile([128, 8, TOK], BF16)    # packed channel-major h
        pending = []                             # deferred normalize work

        def flush_normalize():
            while pending:
                gg, hsgg, rcc = pending.pop(0)
                for gh in range(4):              # head 4*gg+gh
                    s2 = 2 * gg + gh // 2
                    par = gh % 2
                    pgb = psum.tile([64, TOK], F32, tag="pv", bufs=4,
                                    name=f"pgb_{gg}_{gh}")
                    nc.tensor.matmul(pgb, ones_sb[0:1, :], rcc[0:1, gh, :],
                                     start=True, stop=True)
                    hview = hT16[64 * par:64 * par + 64, s2, :]
                    nc.vector.tensor_mul(hview, hsgg[0:64, gh, :], pgb)
                    nc.gpsimd.tensor_tensor(
                        hview, hview,
                        bv_ch[64 * par:64 * par + 64, s2:s2 + 1].to_broadcast(
                            [64, TOK]), ALU.add)
        for s in range(8):                       # head pair (2s, 2s+1)
            g = s // 2
            sg = s % 2
            if sg == 0 and 2 <= g + 1 <= 3:
                kv_recv(g + 1)   # overlaps group g's attention; WAR-safe
            if sg == 0:
                hsg = kv.tile([65, 4, TOK], F32R, tag="hs", name=f"hsg{g}")
            buf = g % 2
            kzs = kz[buf]
            v2s = v2[buf]
            pv_acc = [psum.tile([65, TOK], F32, tag="pv", bufs=4,
                                name=f"pvacc{s}_{i}") for i in range(4)]
            ets = [None] * KT

            def pv_step(kt):
                # PV for k-tile kt (key halves on PE tiles T0/T8)
                et = ets[kt]
                for par in range(2):             # head parity
                    vsl = 65 * (2 * sg + par)
                    nc.tensor.matmul(pv_acc[2 * par],
                                     v2s[0:64, kt, vsl:vsl + 65],
                                     et[0:64, par, :],
                                     start=(kt == 0), stop=(kt == KT - 1))
                    nc.tensor.matmul(pv_acc[2 * par + 1],
                                     v2s[64:128, kt, vsl:vsl + 65],
                                     et[64:128, par, :],
                                     start=(kt == 0), stop=(kt == KT - 1))

            SKEW = 3        # PV trails QK/exp so the PE never waits on ACT
            for kt in range(KT):
                sc = psum.tile([128, 2, TOK], F32, tag="mm", name=f"sc{s}_{kt}")
                nc.tensor.matmul(sc[:, 0, :],
                                 kzs[0:64, sg, 128 * kt:128 * kt + 128],
                                 q_sb[0:64, s, :], start=True, stop=True)
                nc.tensor.matmul(sc[:, 1, :],
                                 kzs[64:128, sg, 128 * kt:128 * kt + 128],
                                 q_sb[64:128, s, :], start=True, stop=True)
                et = kv.tile([128, 2, TOK], BF16, tag="et", bufs=SKEW + 2,
                             name=f"et{s}_{kt}")
                nc.scalar.activation(et, sc, AF.Exp)
                ets[kt] = et
                if kt >= SKEW:
                    pv_step(kt - SKEW)
                if kt == 5 and sg == 0 and pending:
                    flush_normalize()   # previous group's h, off critical path
            for kt in range(KT - SKEW, KT):
                pv_step(kt)

            # drain PV psums: hs = pvT0 + pvT8 into the group slab
            # (gpsimd has no PSUM port: psum-touching ops stay on DVE)
            for par in range(2):
                c8 = kv.tile([65, TOK], F32R, tag="c8", name=f"c8_{s}_{par}")
                nc.vector.tensor_copy(c8, pv_acc[2 * par + 1])
                nc.vector.tensor_tensor(hsg[:, 2 * sg + par, :],
                                        pv_acc[2 * par], c8, ALU.add)
            if sg == 1:
                # one batched reciprocal for the whole group's denominators
                # (DVE reciprocal has a ~2.9us fixed cost per instruction);
                # the pgb matmuls + h writes are DEFERRED into the next
                # group's attention so the PE never waits on this chain
                rc = kv.tile([1, 4, TOK], F32R, tag="rc", name=f"rc_{g}")
                with nc.allow_low_precision(reason="softmax denominator"):
                    nc.vector.reciprocal(rc, hsg[64:65, :, :])
                pending.append((g, hsg, rc))

        flush_normalize()                # last group's h
        # ---- stage 3: routing gates -> per-token scalar g --------------
        prs = psum.tile([17, TOK], F32, tag="pv", bufs=4)
        for s in range(8):
            nc.tensor.matmul(prs, w_rs16[:, s, :], hT16[:, s, :],
                             start=(s == 0), stop=(s == 7))
        rs_sb = const.tile([17, TOK], F32)
        nc.vector.tensor_tensor(rs_sb, prs,
                                brs_sb[:, 0:1].to_broadcast([17, TOK]), ALU.add)
        lg_t = const.tile([128, 4, 17], F32)
        for c4 in range(4):
            pt_ = psum.tile([128, 17], F32, tag="pv", bufs=4, name=f"pt{c4}")
            nc.tensor.transpose(pt_, rs_sb[:, 128 * c4:128 * c4 + 128],
                                ident_sb[0:17, 0:17])
            nc.vector.tensor_copy(lg_t[:, c4, :], pt_)

        e15 = const.tile([128, 4, 15], F32)
        nc.scalar.activation(e15, lg_t[:, :, 0:15], AF.Exp)
        e2 = const.tile([128, 4, 2], F32)
        nc.scalar.activation(e2, lg_t[:, :, 15:17], AF.Exp)
        s15 = const.tile([128, 4, 1], F32)
        nc.vector.tensor_reduce(s15, e15, AX.X, ALU.add)
        s2 = const.tile([128, 4, 1], F32)
        nc.vector.tensor_reduce(s2, e2, AX.X, ALU.add)
        m1 = const.tile([128, 4, 1], F32)
        nc.vector.tensor_reduce(m1, e15, AX.X, ALU.max)
        msk = const.tile([128, 4, 15], F32)
        nc.vector.tensor_tensor(msk, e15, m1.to_broadcast([128, 4, 15]), ALU.is_ge)
        e15b = const.tile([128, 4, 15], F32)
        nc.vector.scalar_tensor_tensor(e15b, msk, -1e30, e15, ALU.mult, ALU.add)
        m2 = const.tile([128, 4, 1], F32)
        nc.vector.tensor_reduce(m2, e15b, AX.X, ALU.max)
        nc.vector.tensor_tensor(msk, e15b, m2.to_broadcast([128, 4, 15]), ALU.is_ge)
        nc.vector.scalar_tensor_tensor(e15b, msk, -1e30, e15b, ALU.mult, ALU.add)
        m3 = const.tile([128, 4, 1], F32)
        nc.vector.tensor_reduce(m3, e15b, AX.X, ALU.max)
        nc.vector.tensor_add(m1, m1, m2)
        nc.vector.tensor_add(m1, m1, m3)       # m1 = top3 sum of e15
        nc.vector.reciprocal(s15, s15)
        nc.vector.reciprocal(s2, s2)
        ga = const.tile([128, 4, 1], F32)
        nc.vector.tensor_mul(ga, e2[:, :, 0:1], s2)
        gb = const.tile([128, 4, 1], F32)
        nc.vector.tensor_mul(gb, e2[:, :, 1:2], s2)
        nc.vector.tensor_mul(gb, gb, m1)
        nc.vector.tensor_mul(gb, gb, s15)
        nc.vector.tensor_scalar_mul(gb, gb, 6.0)
        gsc = const.tile([128, 4, 1], F32)
        nc.vector.scalar_tensor_tensor(gsc, ga, 2.0, gb, ALU.mult, ALU.add)

        # ---- stage 4: output projection --------------------------------
        for nt in range(2):
            po = [psum.tile([128, 2, TOK], F32, tag="mm", name=f"po{nt}_{i}")
                  for i in range(2)]
            for s in range(8):
                for mt in range(4):
                    nc.tensor.matmul(
                        po[mt // 2][:, mt % 2, :],
                        hT16[:, s, 128 * mt:128 * mt + 128],
                        wp_sb[:, s, TOK * nt:TOK * nt + TOK],
                        start=(s == 0), stop=(s == 7))
            for mt in range(4):
                ob = kv.tile([128, TOK], F32, tag="ob", bufs=3,
                             name=f"ob{nt}_{mt}")
                nc.vector.tensor_mul(ob, po[mt // 2][:, mt % 2, :],
                                     gsc[:, mt, 0:1].to_broadcast([128, TOK]))
                nc.vector.tensor_add(ob, ob, bp_rep[:, TOK * nt:TOK * nt + TOK])
                nc.sync.dma_start(
                    out=out[128 * mt:128 * mt + 128, TOK * nt:TOK * nt + TOK],
                    in_=ob)

    nc.compile()
    return nc


_NC_CACHE = {}


def _get_nc():
    if "nc" not in _NC_CACHE:
        _NC_CACHE["nc"] = build_nc()
    return _NC_CACHE["nc"]


def _wdev(W):
    """[out,in] weight -> device layout [p, ks, co]: contiguous per
    partition so the upfront DMA is one descriptor per partition."""
    wT = np.asarray(W, np.float32).T.astype(BF16NP)      # [in, out]
    return np.ascontiguousarray(wT.reshape(8, 128, D).transpose(1, 0, 2))


def _host_prep(x, Wq, bq, Wk, bk, Wv, bv, Wp, bp, Wr, br, Ws, bs,
               temperature, query_embedding):
    f32 = np.float32
    xf = np.ascontiguousarray(x, dtype=f32).reshape(B * N, D)
    shared = {
        "wqT": _wdev(Wq), "wkT": _wdev(Wk), "wvT": _wdev(Wv),
        "wpT": np.ascontiguousarray(np.asarray(Wp, f32).T.astype(BF16NP)),
        "wrsT": np.ascontiguousarray(
            np.concatenate([np.asarray(Wr, f32), np.asarray(Ws, f32)], 0).T),
        "bq": np.ascontiguousarray(bq, f32),
        "bv": np.ascontiguousarray(bv, f32), "bp": np.ascontiguousarray(bp, f32),
        "brs": np.concatenate([np.asarray(br, f32), np.asarray(bs, f32)]),
        "temp16": np.ascontiguousarray(np.asarray(temperature, f32).reshape(H)),
        "qe": np.ascontiguousarray(np.asarray(query_embedding, f32).reshape(H, HD)),
        "ident": np.eye(128, dtype=f32),
        "ones_r": np.ones((128, HD), dtype=f32),
    }
    ch = np.arange(D)
    head_of_ch = ch // HD
    msel = np.zeros((8, 128, 16), BF16NP)
    esel = np.zeros((8, 16, 128), f32)
    for s in range(8):
        hh = head_of_ch[128 * s:128 * s + 128]
        msel[s, np.arange(128), hh] = 1.0
        esel[s, hh, np.arange(128)] = 1.0
    shared["msel"] = msel
    shared["esel"] = esel

    in_maps = []
    for c in range(NCORE):
        rows = slice((c // 4) * N + TOK * (c % 4),
                     (c // 4) * N + TOK * (c % 4) + TOK)
        m = dict(shared)
        m["xT"] = np.ascontiguousarray(
            xf[rows].T.reshape(8, 128, TOK).transpose(1, 0, 2).astype(BF16NP))
        in_maps.append(m)
    return in_maps


def kernel(**inputs):
    nc = _get_nc()
    in_maps = _host_prep(**inputs)
    res = run_bass_kernel_spmd(nc, in_maps, core_ids=list(range(NCORE)))
    shards = [res.results[c]["out"] for c in range(NCORE)]
    return np.concatenate(shards, 0).reshape(B, N, D)
